# revision 31
# baseline (speedup 1.0000x reference)
"""Causal multi-head self-attention (B=2, S=2048, D=1024, H=16) for 8 trn2
NeuronCores.

Sharding: 2-way data parallel over batch x 4-way tensor parallel over heads.
Core c handles batch c//4 and heads [(c%4)*4, (c%4)*4+4) (dh_local=256).
Each core computes its heads' q/k/v projections, causal softmax attention
probabilities (written as the p_attn output shard), the attention context,
and a partial output projection over its 256 head-dims. The host sums the 4
partial output projections per batch (the TP all-reduce) and adds bo.

The upper-triangle (masked) region of p_attn is never written on device: the
runtime hands the kernel zero-initialised output buffers (both the native
run_neff path and the bass2jax/PJRT donation path guarantee this), so the
strictly-causal zeros come for free.

Device-side notes:
  - activations arrive pre-transposed (xT: [D, S]) so every matmul
    contraction dim lands on SBUF partitions.
  - q/k/v are produced transposed ([dh, S]); v is then PE-transposed back to
    [S, dh] (needed as the moving operand of p @ v) and stored fp16.
  - softmax needs no running-max pass: scores/sqrt(D) have |s| < ~1.
  - scores/projection/output matmuls run as float32r (full-rate PE); the
    softmax strip, its PE transposes, and the p @ v matmul run in fp16
    (p_out is written fp16 and widened to fp32 on the host). p @ v uses v
    as the stationary operand and 512-wide transposed-p chunks as the
    moving operand, producing the context directly transposed ([dh, S]).
"""

import os
import numpy as np

B, S, D, H = 2, 2048, 1024, 16
N_CORES = 8
TP = 4              # cores per batch (head-parallel group size)
HL = H // TP        # heads per core = 4
DK = D // H         # 64
DHL = HL * DK       # local head dims per core = 256
P = 128             # SBUF partitions
KO = D // P         # 8 contraction subtiles for the projections
CW = 512            # max chunk width (matmul moving-dim / PSUM bank)
NEG = -30000.0      # additive causal mask (exp -> exact 0 after /32 scale)
SCALE = 1.0 / float(np.sqrt(np.float32(D)))  # faithful sqrt(d_model) scale

_CACHE = {}


def _build_nc():
    import concourse.bass as bass
    import concourse.mybir as mybir
    import concourse.tile as tile
    from concourse import bacc

    f32 = mybir.dt.float32
    f16 = mybir.dt.float16
    mmdt = f16                     # dtype of tensors feeding PE matmuls
    Act = mybir.ActivationFunctionType

    nc = bacc.Bacc(
        "TRN2",
        target_bir_lowering=False,
        debug=False,
        enable_asserts=False,
        num_devices=N_CORES,
    )

    # Per-core inputs.
    xqT = nc.dram_tensor("xqT", [D, S], mmdt, kind="ExternalInput").ap()
    xkT = nc.dram_tensor("xkT", [D, S], mmdt, kind="ExternalInput").ap()
    xvT = nc.dram_tensor("xvT", [D, S], mmdt, kind="ExternalInput").ap()
    wqT = nc.dram_tensor("wqT", [D, DHL], mmdt, kind="ExternalInput").ap()
    wkT = nc.dram_tensor("wkT", [D, DHL], mmdt, kind="ExternalInput").ap()
    wvT = nc.dram_tensor("wvT", [D, DHL], mmdt, kind="ExternalInput").ap()
    woT = nc.dram_tensor("woT", [DHL, D], mmdt, kind="ExternalInput").ap()
    bqkv = nc.dram_tensor("bqkv", [3, DHL], f32, kind="ExternalInput").ap()
    mask = nc.dram_tensor("mask", [P, P], f16, kind="ExternalInput").ap()
    ident = nc.dram_tensor("ident", [P, P], f16, kind="ExternalInput").ap()
    ident32 = nc.dram_tensor("ident32", [P, P], f32, kind="ExternalInput").ap()

    # Per-core outputs.
    p_out = nc.dram_tensor("p_out", [HL, S, S], f16, kind="ExternalOutput").ap()
    o_out = nc.dram_tensor("o_out", [S, D], f32, kind="ExternalOutput").ap()

    def tr(out_ap, in_ap, ident_ap):
        nc.tensor.transpose(out_ap, in_ap, ident_ap)

    with tile.TileContext(nc) as tc:
        with (
            tc.tile_pool(name="consts", bufs=1) as consts,
            tc.tile_pool(name="xin", bufs=3) as xin,
            tc.tile_pool(name="big", bufs=1) as big,
            tc.tile_pool(name="vt", bufs=2) as vtp,
            tc.tile_pool(name="pstrip", bufs=6) as pstrip,
            tc.tile_pool(name="ptile", bufs=4) as ptile,
            tc.tile_pool(name="outp", bufs=2) as outp,
            tc.tile_pool(name="small", bufs=6) as small,
            tc.tile_pool(name="ps", bufs=2, space="PSUM") as psp,
            tc.tile_pool(name="pstr", bufs=2, space="PSUM") as pstr,
            tc.tile_pool(name="psctx", bufs=1, space="PSUM") as psctx,
            tc.tile_pool(name="psr", bufs=1, space="PSUM") as psrp,
        ):
            # ---- constants -------------------------------------------------
            ident_sb = consts.tile([P, P], f16, tag="ident")
            nc.sync.dma_start(ident_sb[:], ident)
            ident32_sb = consts.tile([P, P], f32, tag="ident32")
            nc.sync.dma_start(ident32_sb[:], ident32)
            mask_sb = consts.tile([P, P], f16, tag="mask")
            nc.sync.dma_start(mask_sb[:], mask)
            wq_sb = consts.tile([P, KO, DHL], mmdt, tag="wq")
            nc.sync.dma_start(wq_sb[:], wqT.rearrange("(ko ki) m -> ki ko m", ki=P))
            wk_sb = consts.tile([P, KO, DHL], mmdt, tag="wk")
            nc.sync.dma_start(wk_sb[:], wkT.rearrange("(ko ki) m -> ki ko m", ki=P))
            wv_sb = consts.tile([P, KO, DHL], mmdt, tag="wv")
            nc.sync.dma_start(wv_sb[:], wvT.rearrange("(ko ki) m -> ki ko m", ki=P))
            wo_sb = consts.tile([P, DHL // P, D], mmdt, tag="wo")
            nc.sync.dma_start(wo_sb[:], woT.rearrange("(ko ki) n -> ki ko n", ki=P))
            bias_sb = consts.tile([P, 6], f32, tag="bias")
            nc.sync.dma_start(bias_sb[:], bqkv.rearrange("t (o p) -> p (t o)", p=P))

            # ---- phase P: projections -------------------------------------
            # Per-(mb, chunk) tiles so attention strips depend only on the
            # projection chunks they actually read (q-group qg reads chunk qg
            # of qT, chunks 0..qg of kT and v) instead of the whole tensors.
            qTt = [[big.tile([P, CW], mmdt, tag=f"qT{mb}{ch}",
                              name=f"qT{mb}{ch}") for ch in range(4)]
                   for mb in range(2)]
            kTt = [[big.tile([P, CW], mmdt, tag=f"kT{mb}{ch}",
                              name=f"kT{mb}{ch}") for ch in range(4)]
                   for mb in range(2)]
            vtile = [big.tile([P, 4, DHL], f16, tag=f"v{ch}", name=f"v{ch}")
                     for ch in range(4)]

            for nch in range(S // CW):
                for t, (xT, w_sb) in enumerate(((xqT, wq_sb), (xkT, wk_sb),
                                                (xvT, wv_sb))):
                    xT_r = xT.rearrange("(ko ki) s -> ki ko s", ki=P)
                    x_t = xin.tile([P, KO, CW], mmdt, tag="x")
                    nc.sync.dma_start(x_t[:], xT_r[:, :, nch * CW:(nch + 1) * CW])
                    v_t = None if t < 2 else vtp.tile([P, 2, CW], f16, tag="vt")
                    ps = psp.tile([P, 2 * CW], f32, tag="ps")
                    for mb in range(2):
                        for ko in range(KO):
                            nc.tensor.matmul(
                                ps[:, mb * CW:mb * CW + CW],
                                w_sb[:, ko, mb * P:(mb + 1) * P],
                                x_t[:, ko, :],
                                start=(ko == 0),
                                stop=(ko == KO - 1),
                            )
                        tgt = ((qTt, kTt)[t][mb][nch][:, :] if t < 2
                               else v_t[:, mb, :])
                        nc.scalar.activation(
                            tgt, ps[:, mb * CW:mb * CW + CW], Act.Identity,
                            bias=bias_sb[:, t * 2 + mb:t * 2 + mb + 1],
                        )
                    if t == 2:
                        # vT chunk -> v[S, dh] via grouped PE transposes (f16)
                        for mb in range(2):
                            pst = pstr.tile([P, CW], f16, tag="pst")
                            for j in range(CW // P):
                                tr(pst[:, j * P:(j + 1) * P],
                                   v_t[:, mb, j * P:(j + 1) * P], ident_sb[:])
                            nc.vector.tensor_copy(
                                vtile[nch][:, :, mb * P:(mb + 1) * P],
                                pst[:].rearrange("p (a b) -> p a b", a=4),
                            )

            # ---- phase A+O: attention, interleaved output projection ------
            # q-group outer so each 512-query group's ctxT (all heads) closes
            # early and its output projection overlaps the next group.
            for qg in range(4):
                nkb = 4 * qg + 4
                ctxT_g = big.tile([P, DHL // P, CW], mmdt, tag=f"ctxT{qg}")
                for h in range(HL):
                    mb, off = h // 2, (h % 2) * DK
                    # transposed-p chunks [key-in-block, kb, q-in-group], fp16
                    # (no zero-fill needed: the p~ @ v matmuls below read
                    # only cols >= q0 of each key block, all of which the
                    # strip transposes write)
                    pT_b = ptile.tile([P, S // P, CW], f16, tag="pT")
                    rec4 = small.tile([P, 4], f32, tag="rec4")
                    for j in range(4):
                        qb = 4 * qg + j
                        kw = (qb + 1) * P
                        p_t = pstrip.tile([P, S], f16, tag="p")
                        sums_t = small.tile([P, 4], f32, tag="sums")
                        nc2 = (kw + 2 * CW - 1) // (2 * CW)
                        for c in range(nc2):
                            lo = c * 2 * CW
                            w2 = min(2 * CW, kw - lo)
                            ps = psp.tile([P, 2 * CW], f32, tag="ps")
                            for u in range(0, w2, CW):
                                wu = min(CW, w2 - u)
                                nc.tensor.matmul(
                                    ps[:, u:u + wu],
                                    qTt[mb][qg][off:off + DK,
                                                (qb % 4) * P:(qb % 4 + 1) * P],
                                    kTt[mb][(lo + u) // CW][off:off + DK, 0:wu],
                                    start=True, stop=True,
                                )
                            if c == nc2 - 1:
                                # causal mask accumulated on the PE itself:
                                # ps[:, -128:] += ident.T @ mask  (no DVE sync)
                                nc.tensor.matmul(
                                    ps[:, w2 - P:w2], ident_sb[:], mask_sb[:],
                                    start=False, stop=True,
                                    skip_group_check=True,
                                )
                            nc.scalar.activation(
                                p_t[:, lo:lo + w2], ps[:, :w2], Act.Exp,
                                scale=SCALE, accum_out=sums_t[:, c:c + 1],
                            )
                        nc.vector.reduce_sum(
                            out=sums_t[:, 3:4], in_=sums_t[:, 0:nc2],
                            axis=mybir.AxisListType.X,
                        )
                        nc.vector.reciprocal(rec4[:, j:j + 1], sums_t[:, 3:4])
                        # transpose the UNNORMALIZED strip into the pT chunks
                        # (in-place normalize below is WAR-ordered after these)
                        for g2 in range(0, (kw + CW - 1) // CW, 2):
                            lo = g2 * CW
                            w = min(2 * CW, kw - lo)
                            nb = w // P
                            pst = pstr.tile([P, 2 * CW], f16, tag="pst")
                            for jj in range(nb):
                                tr(pst[:, jj * P:(jj + 1) * P],
                                   p_t[:, lo + jj * P:lo + (jj + 1) * P],
                                   ident_sb[:])
                            nc.vector.tensor_copy(
                                pT_b[:, g2 * 4:g2 * 4 + nb, j * P:(j + 1) * P],
                                pst[:, :w].rearrange("p (a b) -> p a b", a=nb),
                            )
                        # normalize for the p_attn output; DMA split across
                        # the Sync and GpSimd queues
                        if j % 2 == 0:
                            nc.vector.tensor_scalar_mul(
                                p_t[:, 0:kw], p_t[:, 0:kw], rec4[:, j:j + 1]
                            )
                        else:
                            nc.scalar.mul(
                                p_t[:, 0:kw], p_t[:, 0:kw], rec4[:, j:j + 1]
                            )
                        eng = nc.sync if j % 2 == 0 else nc.gpsimd
                        eng.dma_start(
                            p_out[h, qb * P:(qb + 1) * P, 0:kw], p_t[:, 0:kw]
                        )
                    # recipT: [128,4] strips-recips -> broadcast row [64,512]
                    psr = psrp.tile([1, CW], f32, tag="psr")
                    for jj in range(4):
                        nc.tensor.transpose(
                            psr[0:1, jj * P:(jj + 1) * P],
                            rec4[:, jj:jj + 1], ident32_sb[:],
                        )
                    recT = small.tile([1, CW], f32, tag="recT")
                    nc.vector.tensor_copy(recT[0:1, :], psr[0:1, :])
                    recB = small.tile([DK, CW], f32, tag="recB")
                    nc.gpsimd.partition_broadcast(recB[:], recT[0:1, :])
                    # p~ @ v for the q-group: ctxT[dk, 512q] over key blocks,
                    # v stationary / pT moving; scaled by recipT on copy-out.
                    psc = psctx.tile([DK, CW], f32, tag="psc")
                    for kb in range(nkb):
                        q0 = max(0, kb * P - qg * 4 * P)  # first unmasked col
                        nc.tensor.matmul(
                            psc[:, q0:],
                            vtile[kb // 4][:, kb % 4, h * DK:(h + 1) * DK],
                            pT_b[:, kb, q0:],
                            start=(kb == 0), stop=(kb == nkb - 1),
                            skip_group_check=True,
                        )
                    nc.vector.scalar_tensor_tensor(
                        ctxT_g[off:off + DK, mb, :], psc[:], 1.0, recB[:],
                        op0=mybir.AluOpType.mult, op1=mybir.AluOpType.mult,
                    )
                # output projection for this q-group's 4 s-blocks
                for j in range(4):
                    sb = qg * 4 + j
                    o_t = outp.tile([P, D], f32, tag="o")
                    ps = psp.tile([P, 2 * CW], f32, tag="ps")
                    for nh in range(D // CW):
                        for mbd in range(DHL // P):
                            nc.tensor.matmul(
                                ps[:, nh * CW:(nh + 1) * CW],
                                ctxT_g[:, mbd, j * P:(j + 1) * P],
                                wo_sb[:, mbd, nh * CW:(nh + 1) * CW],
                                start=(mbd == 0), stop=(mbd == DHL // P - 1),
                            )
                    nc.scalar.copy(o_t[:], ps[:])
                    nc.sync.dma_start(o_out[sb * P:(sb + 1) * P, :], o_t[:])

    nc.compile()
    return nc


def _get_nc():
    if "nc" not in _CACHE:
        _CACHE["nc"] = _build_nc()
    return _CACHE["nc"]


def _host_masks():
    # Additive causal mask for the 128x128 diagonal block.
    qr = np.arange(P)[:, None]
    j = np.arange(P)[None, :]
    return np.where(j <= qr, np.float16(0.0), np.float16(NEG))


def kernel(query, key, value, Wq, bq, Wk, bk, Wv, bv, Wo, bo):
    from concourse.bass_utils import run_bass_kernel_spmd

    nc = _get_nc()

    query = np.asarray(query, np.float32)
    key = np.asarray(key, np.float32)
    value = np.asarray(value, np.float32)
    WqT = np.ascontiguousarray(np.asarray(Wq, np.float32).T.astype(np.float16))
    WkT = np.ascontiguousarray(np.asarray(Wk, np.float32).T.astype(np.float16))
    WvT = np.ascontiguousarray(np.asarray(Wv, np.float32).T.astype(np.float16))
    WoT = np.ascontiguousarray(np.asarray(Wo, np.float32).T.astype(np.float16))
    bq = np.asarray(bq, np.float32)
    bk = np.asarray(bk, np.float32)
    bv = np.asarray(bv, np.float32)
    bo = np.asarray(bo, np.float32)

    mask = _host_masks()
    ident = np.eye(P, dtype=np.float16)
    ident32 = np.eye(P, dtype=np.float32)

    in_maps = []
    for c in range(N_CORES):
        b, g = c // TP, c % TP
        cols = slice(g * DHL, (g + 1) * DHL)   # head dims owned by this core
        in_maps.append({
            "xqT": np.ascontiguousarray(query[b].T.astype(np.float16)),
            "xkT": np.ascontiguousarray(key[b].T.astype(np.float16)),
            "xvT": np.ascontiguousarray(value[b].T.astype(np.float16)),
            "wqT": np.ascontiguousarray(WqT[:, cols]),
            "wkT": np.ascontiguousarray(WkT[:, cols]),
            "wvT": np.ascontiguousarray(WvT[:, cols]),
            "woT": np.ascontiguousarray(WoT[cols, :]),
            "bqkv": np.ascontiguousarray(np.stack([bq[cols], bk[cols], bv[cols]])),
            "mask": mask,
            "ident": ident,
            "ident32": ident32,
        })

    res = run_bass_kernel_spmd(
        nc, in_maps, core_ids=list(range(N_CORES)),
        trace=bool(int(os.environ.get("KERNEL_TRACE", "0"))),
    )
    _CACHE["last_result"] = res

    p_attn = np.empty((B, H, S, S), np.float32)
    out = np.empty((B, S, D), np.float32)
    for b in range(B):
        acc = None
        for g in range(TP):
            rr = res.results[b * TP + g]
            p_attn[b, g * HL:(g + 1) * HL] = rr["p_out"].astype(np.float32)
            acc = rr["o_out"] if acc is None else acc + rr["o_out"]
        out[b] = acc + bo
    return out, p_attn


# revision 32
# speedup vs baseline: 1.0310x; 1.0310x over previous
"""Causal multi-head self-attention (B=2, S=2048, D=1024, H=16) for 8 trn2
NeuronCores.

Sharding: 2-way data parallel over batch x 4-way tensor parallel over heads.
Core c handles batch c//4 and heads [(c%4)*4, (c%4)*4+4) (dh_local=256).
Each core computes its heads' q/k/v projections, causal softmax attention
probabilities (written as the p_attn output shard), the attention context,
and a partial output projection over its 256 head-dims. The host sums the 4
partial output projections per batch (the TP all-reduce) and adds bo.

The upper-triangle (masked) region of p_attn is never written on device: the
runtime hands the kernel zero-initialised output buffers (both the native
run_neff path and the bass2jax/PJRT donation path guarantee this), so the
strictly-causal zeros come for free.

Device-side notes:
  - activations arrive pre-transposed (xT: [D, S]) so every matmul
    contraction dim lands on SBUF partitions.
  - q/k/v are produced transposed ([dh, S]); v is then PE-transposed back to
    [S, dh] (needed as the moving operand of p @ v) and stored fp16.
  - softmax needs no running-max pass: scores/sqrt(D) have |s| < ~1.
  - scores/projection/output matmuls run as float32r (full-rate PE); the
    softmax strip, its PE transposes, and the p @ v matmul run in fp16
    (p_out is written fp16 and widened to fp32 on the host). p @ v uses v
    as the stationary operand and 512-wide transposed-p chunks as the
    moving operand, producing the context directly transposed ([dh, S]).
"""

import os
import numpy as np

B, S, D, H = 2, 2048, 1024, 16
N_CORES = 8
TP = 4              # cores per batch (head-parallel group size)
HL = H // TP        # heads per core = 4
DK = D // H         # 64
DHL = HL * DK       # local head dims per core = 256
P = 128             # SBUF partitions
KO = D // P         # 8 contraction subtiles for the projections
CW = 512            # max chunk width (matmul moving-dim / PSUM bank)
NEG = -30000.0      # additive causal mask (exp -> exact 0 after /32 scale)
SCALE = 1.0 / float(np.sqrt(np.float32(D)))  # faithful sqrt(d_model) scale

_CACHE = {}


def _build_nc():
    import concourse.bass as bass
    import concourse.mybir as mybir
    import concourse.tile as tile
    from concourse import bacc

    f32 = mybir.dt.float32
    f16 = mybir.dt.float16
    mmdt = f16                     # dtype of tensors feeding PE matmuls
    Act = mybir.ActivationFunctionType

    nc = bacc.Bacc(
        "TRN2",
        target_bir_lowering=False,
        debug=False,
        enable_asserts=False,
        num_devices=N_CORES,
    )

    # Per-core inputs.
    xqT = nc.dram_tensor("xqT", [D, S], mmdt, kind="ExternalInput").ap()
    xkT = nc.dram_tensor("xkT", [D, S], mmdt, kind="ExternalInput").ap()
    xvT = nc.dram_tensor("xvT", [D, S], mmdt, kind="ExternalInput").ap()
    wqT = nc.dram_tensor("wqT", [D, DHL], mmdt, kind="ExternalInput").ap()
    wkT = nc.dram_tensor("wkT", [D, DHL], mmdt, kind="ExternalInput").ap()
    wvT = nc.dram_tensor("wvT", [D, DHL], mmdt, kind="ExternalInput").ap()
    woT = nc.dram_tensor("woT", [DHL, D], mmdt, kind="ExternalInput").ap()
    bqkv = nc.dram_tensor("bqkv", [3, DHL], f32, kind="ExternalInput").ap()
    mask = nc.dram_tensor("mask", [P, P], f16, kind="ExternalInput").ap()
    ident = nc.dram_tensor("ident", [P, P], f16, kind="ExternalInput").ap()
    ident32 = nc.dram_tensor("ident32", [P, P], f32, kind="ExternalInput").ap()

    # Per-core outputs.
    p_out = nc.dram_tensor("p_out", [HL, S, S], f16, kind="ExternalOutput").ap()
    o_out = nc.dram_tensor("o_out", [S, D], f32, kind="ExternalOutput").ap()

    def tr(out_ap, in_ap, ident_ap):
        nc.tensor.transpose(out_ap, in_ap, ident_ap)

    with tile.TileContext(nc) as tc:
        with (
            tc.tile_pool(name="consts", bufs=1) as consts,
            tc.tile_pool(name="xin", bufs=3) as xin,
            tc.tile_pool(name="big", bufs=1) as big,
            tc.tile_pool(name="vt", bufs=2) as vtp,
            tc.tile_pool(name="pstrip", bufs=4) as pstrip,
            tc.tile_pool(name="ptile", bufs=3) as ptile,
            tc.tile_pool(name="outp", bufs=2) as outp,
            tc.tile_pool(name="small", bufs=6) as small,
            tc.tile_pool(name="ps", bufs=2, space="PSUM") as psp,
            tc.tile_pool(name="pstr", bufs=2, space="PSUM") as pstr,
            tc.tile_pool(name="psctx", bufs=1, space="PSUM") as psctx,
            tc.tile_pool(name="psr", bufs=1, space="PSUM") as psrp,
        ):
            # ---- constants -------------------------------------------------
            ident_sb = consts.tile([P, P], f16, tag="ident")
            nc.sync.dma_start(ident_sb[:], ident)
            ident32_sb = consts.tile([P, P], f32, tag="ident32")
            nc.sync.dma_start(ident32_sb[:], ident32)
            mask_sb = consts.tile([P, P], f16, tag="mask")
            nc.sync.dma_start(mask_sb[:], mask)
            wq_sb = consts.tile([P, KO, DHL], mmdt, tag="wq")
            nc.sync.dma_start(wq_sb[:], wqT.rearrange("(ko ki) m -> ki ko m", ki=P))
            wk_sb = consts.tile([P, KO, DHL], mmdt, tag="wk")
            nc.sync.dma_start(wk_sb[:], wkT.rearrange("(ko ki) m -> ki ko m", ki=P))
            wv_sb = consts.tile([P, KO, DHL], mmdt, tag="wv")
            nc.sync.dma_start(wv_sb[:], wvT.rearrange("(ko ki) m -> ki ko m", ki=P))
            wo_sb = consts.tile([P, DHL // P, D], mmdt, tag="wo")
            nc.sync.dma_start(wo_sb[:], woT.rearrange("(ko ki) n -> ki ko n", ki=P))
            bias_sb = consts.tile([P, 6], f32, tag="bias")
            nc.sync.dma_start(bias_sb[:], bqkv.rearrange("t (o p) -> p (t o)", p=P))

            # ---- phase P: projections -------------------------------------
            # Per-(mb, chunk) tiles so attention strips depend only on the
            # projection chunks they actually read (q-group qg reads chunk qg
            # of qT, chunks 0..qg of kT and v) instead of the whole tensors.
            qTt = [[big.tile([P, CW], mmdt, tag=f"qT{mb}{ch}",
                              name=f"qT{mb}{ch}") for ch in range(4)]
                   for mb in range(2)]
            kTt = [[big.tile([P, CW], mmdt, tag=f"kT{mb}{ch}",
                              name=f"kT{mb}{ch}") for ch in range(4)]
                   for mb in range(2)]
            vtile = [big.tile([P, 4, DHL], f16, tag=f"v{ch}", name=f"v{ch}")
                     for ch in range(4)]

            for nch in range(S // CW):
                for t, (xT, w_sb) in enumerate(((xqT, wq_sb), (xkT, wk_sb),
                                                (xvT, wv_sb))):
                    xT_r = xT.rearrange("(ko ki) s -> ki ko s", ki=P)
                    x_t = xin.tile([P, KO, CW], mmdt, tag="x")
                    nc.sync.dma_start(x_t[:], xT_r[:, :, nch * CW:(nch + 1) * CW])
                    v_t = None if t < 2 else vtp.tile([P, 2, CW], f16, tag="vt")
                    ps = psp.tile([P, 2 * CW], f32, tag="ps")
                    for mb in range(2):
                        for ko in range(KO):
                            nc.tensor.matmul(
                                ps[:, mb * CW:mb * CW + CW],
                                w_sb[:, ko, mb * P:(mb + 1) * P],
                                x_t[:, ko, :],
                                start=(ko == 0),
                                stop=(ko == KO - 1),
                            )
                        tgt = ((qTt, kTt)[t][mb][nch][:, :] if t < 2
                               else v_t[:, mb, :])
                        nc.scalar.activation(
                            tgt, ps[:, mb * CW:mb * CW + CW], Act.Identity,
                            bias=bias_sb[:, t * 2 + mb:t * 2 + mb + 1],
                        )
                    if t == 2:
                        # vT chunk -> v[S, dh] via grouped PE transposes (f16)
                        for mb in range(2):
                            pst = pstr.tile([P, CW], f16, tag="pst")
                            for j in range(CW // P):
                                tr(pst[:, j * P:(j + 1) * P],
                                   v_t[:, mb, j * P:(j + 1) * P], ident_sb[:])
                            nc.vector.tensor_copy(
                                vtile[nch][:, :, mb * P:(mb + 1) * P],
                                pst[:].rearrange("p (a b) -> p a b", a=4),
                            )

            # ---- phase A+O: attention, interleaved output projection ------
            # q-group outer so each 512-query group's ctxT (all heads) closes
            # early and its output projection overlaps the next group.
            for qg in range(4):
                nkb = 4 * qg + 4
                ctxT_g = big.tile([P, DHL // P, CW], mmdt, tag=f"ctxT{qg}")
                for h in range(HL):
                    mb, off = h // 2, (h % 2) * DK
                    # transposed-p chunks [key-in-block, kb, q-in-group], fp16
                    # (no zero-fill needed: the p~ @ v matmuls below read
                    # only cols >= q0 of each key block, all of which the
                    # strip transposes write)
                    pT_b = ptile.tile([P, S // P, CW], f16, tag="pT")
                    rec4 = small.tile([P, 4], f32, tag="rec4")
                    for j in range(4):
                        qb = 4 * qg + j
                        kw = (qb + 1) * P
                        p_t = pstrip.tile([P, S], f16, tag="p")
                        sums_t = small.tile([P, 4], f32, tag="sums")
                        nc2 = (kw + 2 * CW - 1) // (2 * CW)
                        for c in range(nc2):
                            lo = c * 2 * CW
                            w2 = min(2 * CW, kw - lo)
                            ps = psp.tile([P, 2 * CW], f32, tag="ps")
                            for u in range(0, w2, CW):
                                wu = min(CW, w2 - u)
                                nc.tensor.matmul(
                                    ps[:, u:u + wu],
                                    qTt[mb][qg][off:off + DK,
                                                (qb % 4) * P:(qb % 4 + 1) * P],
                                    kTt[mb][(lo + u) // CW][off:off + DK, 0:wu],
                                    start=True, stop=True,
                                )
                            if c == nc2 - 1:
                                # causal mask accumulated on the PE itself:
                                # ps[:, -128:] += ident.T @ mask  (no DVE sync)
                                nc.tensor.matmul(
                                    ps[:, w2 - P:w2], ident_sb[:], mask_sb[:],
                                    start=False, stop=True,
                                    skip_group_check=True,
                                )
                            nc.scalar.activation(
                                p_t[:, lo:lo + w2], ps[:, :w2], Act.Exp,
                                scale=SCALE, accum_out=sums_t[:, c:c + 1],
                            )
                        nc.vector.reduce_sum(
                            out=sums_t[:, 3:4], in_=sums_t[:, 0:nc2],
                            axis=mybir.AxisListType.X,
                        )
                        nc.vector.reciprocal(rec4[:, j:j + 1], sums_t[:, 3:4])
                        # transpose the UNNORMALIZED strip into the pT chunks
                        # (in-place normalize below is WAR-ordered after these)
                        for g2 in range(0, (kw + CW - 1) // CW, 2):
                            lo = g2 * CW
                            w = min(2 * CW, kw - lo)
                            nb = w // P
                            pst = pstr.tile([P, 2 * CW], f16, tag="pst")
                            for jj in range(nb):
                                tr(pst[:, jj * P:(jj + 1) * P],
                                   p_t[:, lo + jj * P:lo + (jj + 1) * P],
                                   ident_sb[:])
                            dst_ap = pT_b[:, g2 * 4:g2 * 4 + nb, j * P:(j + 1) * P]
                            src_ap = pst[:, :w].rearrange("p (a b) -> p a b", a=nb)
                            if (j + g2 // 2) % 2 == 0:
                                nc.vector.tensor_copy(dst_ap, src_ap)
                            else:
                                nc.scalar.copy(dst_ap, src_ap)
                        # normalize for the p_attn output; DMA split across
                        # the Sync and GpSimd queues
                        nc.vector.tensor_scalar_mul(
                            p_t[:, 0:kw], p_t[:, 0:kw], rec4[:, j:j + 1]
                        )
                        eng = nc.sync if j % 2 == 0 else nc.gpsimd
                        eng.dma_start(
                            p_out[h, qb * P:(qb + 1) * P, 0:kw], p_t[:, 0:kw]
                        )
                    # recipT: [128,4] strips-recips -> broadcast row [64,512]
                    psr = psrp.tile([1, CW], f32, tag="psr")
                    for jj in range(4):
                        nc.tensor.transpose(
                            psr[0:1, jj * P:(jj + 1) * P],
                            rec4[:, jj:jj + 1], ident32_sb[:],
                        )
                    recT = small.tile([1, CW], f32, tag="recT")
                    nc.vector.tensor_copy(recT[0:1, :], psr[0:1, :])
                    recB = small.tile([DK, CW], f32, tag="recB")
                    nc.gpsimd.partition_broadcast(recB[:], recT[0:1, :])
                    # p~ @ v for the q-group: ctxT[dk, 512q] over key blocks,
                    # v stationary / pT moving; scaled by recipT on copy-out.
                    psc = psctx.tile([DK, CW], f32, tag="psc")
                    for kb in range(nkb):
                        q0 = max(0, kb * P - qg * 4 * P)  # first unmasked col
                        nc.tensor.matmul(
                            psc[:, q0:],
                            vtile[kb // 4][:, kb % 4, h * DK:(h + 1) * DK],
                            pT_b[:, kb, q0:],
                            start=(kb == 0), stop=(kb == nkb - 1),
                            skip_group_check=True,
                        )
                    nc.vector.scalar_tensor_tensor(
                        ctxT_g[off:off + DK, mb, :], psc[:], 1.0, recB[:],
                        op0=mybir.AluOpType.mult, op1=mybir.AluOpType.mult,
                    )
                # output projection for this q-group's 4 s-blocks
                for j in range(4):
                    sb = qg * 4 + j
                    o_t = outp.tile([P, D], f32, tag="o")
                    ps = psp.tile([P, 2 * CW], f32, tag="ps")
                    for nh in range(D // CW):
                        for mbd in range(DHL // P):
                            nc.tensor.matmul(
                                ps[:, nh * CW:(nh + 1) * CW],
                                ctxT_g[:, mbd, j * P:(j + 1) * P],
                                wo_sb[:, mbd, nh * CW:(nh + 1) * CW],
                                start=(mbd == 0), stop=(mbd == DHL // P - 1),
                            )
                    nc.scalar.copy(o_t[:], ps[:])
                    nc.sync.dma_start(o_out[sb * P:(sb + 1) * P, :], o_t[:])

    nc.compile()
    return nc


def _get_nc():
    if "nc" not in _CACHE:
        _CACHE["nc"] = _build_nc()
    return _CACHE["nc"]


def _host_masks():
    # Additive causal mask for the 128x128 diagonal block.
    qr = np.arange(P)[:, None]
    j = np.arange(P)[None, :]
    return np.where(j <= qr, np.float16(0.0), np.float16(NEG))


def kernel(query, key, value, Wq, bq, Wk, bk, Wv, bv, Wo, bo):
    from concourse.bass_utils import run_bass_kernel_spmd

    nc = _get_nc()

    query = np.asarray(query, np.float32)
    key = np.asarray(key, np.float32)
    value = np.asarray(value, np.float32)
    WqT = np.ascontiguousarray(np.asarray(Wq, np.float32).T.astype(np.float16))
    WkT = np.ascontiguousarray(np.asarray(Wk, np.float32).T.astype(np.float16))
    WvT = np.ascontiguousarray(np.asarray(Wv, np.float32).T.astype(np.float16))
    WoT = np.ascontiguousarray(np.asarray(Wo, np.float32).T.astype(np.float16))
    bq = np.asarray(bq, np.float32)
    bk = np.asarray(bk, np.float32)
    bv = np.asarray(bv, np.float32)
    bo = np.asarray(bo, np.float32)

    mask = _host_masks()
    ident = np.eye(P, dtype=np.float16)
    ident32 = np.eye(P, dtype=np.float32)

    in_maps = []
    for c in range(N_CORES):
        b, g = c // TP, c % TP
        cols = slice(g * DHL, (g + 1) * DHL)   # head dims owned by this core
        in_maps.append({
            "xqT": np.ascontiguousarray(query[b].T.astype(np.float16)),
            "xkT": np.ascontiguousarray(key[b].T.astype(np.float16)),
            "xvT": np.ascontiguousarray(value[b].T.astype(np.float16)),
            "wqT": np.ascontiguousarray(WqT[:, cols]),
            "wkT": np.ascontiguousarray(WkT[:, cols]),
            "wvT": np.ascontiguousarray(WvT[:, cols]),
            "woT": np.ascontiguousarray(WoT[cols, :]),
            "bqkv": np.ascontiguousarray(np.stack([bq[cols], bk[cols], bv[cols]])),
            "mask": mask,
            "ident": ident,
            "ident32": ident32,
        })

    res = run_bass_kernel_spmd(
        nc, in_maps, core_ids=list(range(N_CORES)),
        trace=bool(int(os.environ.get("KERNEL_TRACE", "0"))),
    )
    _CACHE["last_result"] = res

    p_attn = np.empty((B, H, S, S), np.float32)
    out = np.empty((B, S, D), np.float32)
    for b in range(B):
        acc = None
        for g in range(TP):
            rr = res.results[b * TP + g]
            p_attn[b, g * HL:(g + 1) * HL] = rr["p_out"].astype(np.float32)
            acc = rr["o_out"] if acc is None else acc + rr["o_out"]
        out[b] = acc + bo
    return out, p_attn
